# revision 69
# baseline (speedup 1.0000x reference)
"""Trainium2 Bass kernel for causal GQA self-attention (8 docs x 1024 tokens,
dim 1024, 16 q heads / 4 kv heads, head_dim 64, RMS-normed+RoPE q/k).

Sharding: data-parallel over docs — core c computes doc c end to end.

v3 design notes (cost-model driven):
  - QKV/V projections and the output projection run in fp8-e4m3 DoubleRow
    mode (0.5 cycles/row): operands are split into hi+lo e4m3 parts (weights
    pre-scaled by 32 so the lo residual stays out of the subnormal range);
    slot-paired products hi*hi + (hi*lo + lo*hi) give better-than-bf16
    accuracy at 0.75x the bf16 PE cost.  Dequant rides psum->sbuf copies.
  - psum->sbuf copies of q/k (and the final output) ride the Activation
    engine, which is otherwise idle outside the exp stream.
  - k is never scaled on chip: s_k = rsqrt(mean k^2) is folded into the
    exp() activation's per-partition scale operand (scores^T layout has tk
    on partitions); k sum-squares are computed directly in column form by
    tiny ones-vector matmuls.  No causal-mask PE matmuls: the diagonal
    128x128 block of each P tile is multiplied by a 0/1 mask on DVE post-exp.
  - the attention phase is Activation-bound (exp stream ~80us), so all
    other PE work that can move there does: v projections, q-scale
    finishes, per-pair normalization, and the hi/lo split of y for the
    fp8 output projection are interleaved between head pairs.
  - one shared [128,512] psum ring (3 banks) serves projection halves,
    rope, v, scale broadcasts and PV; scores use a 2-deep [128,1024] ring.
"""

import os
import sys

sys.path.insert(0, "/opt/trn_rl_repo")

import numpy as np
import ml_dtypes

import concourse.bass as bass
import concourse.bacc as bacc
import concourse.mybir as mybir
import concourse.tile as tile
from concourse import bass_utils
from contextlib import ExitStack

f32 = mybir.dt.float32
f32r = mybir.dt.float32r
bf16 = mybir.dt.bfloat16
fp8 = mybir.dt.float8e4
BF = ml_dtypes.bfloat16
F8 = ml_dtypes.float8_e4m3fn
DR = mybir.MatmulPerfMode.DoubleRow

DIM = 1024
H = 16
HKV = 4
HD = 64
B = 8
S = 1024
NC = 8          # d chunks of 128
WSC = 32.0      # fp8 weight pre-scale
EPS = float(np.finfo(np.float32).eps)
Exp = mybir.ActivationFunctionType.Exp
Sqrt = mybir.ActivationFunctionType.Sqrt
Copy = mybir.ActivationFunctionType.Copy

# aux blob column offsets (all [128, n] bf16)
A_COS, A_SIN, A_RT, A_BSQ = 0, 1024, 2048, 2176
A_ID, A_MTS, A_ONE = 2304, 2432, 2560
A_N = 2561
_CACHE = {}


def _build():
    nc = bacc.Bacc("TRN2")
    inp = {}
    for name, shape, dt in [
        ("xf", [128, NC * 2 * S], fp8),         # per kc: [lo S | hi S]
        ("wqk", [128, 10 * NC * 256], fp8),     # per oc: kc-major [hi | lo]
        ("wv", [128, NC * 512], fp8),           # per kc: [hi 256 | lo 256]
        ("wp", [128, NC * 2 * DIM], fp8),       # per dc: kc-major [hi | lo]
        ("aux", [128, A_N], bf16),
        ("b2", [2, 128], f32r),
        ("gains", [8, 2], f32),
    ]:
        inp[name] = nc.dram_tensor(name, shape, dt, kind="ExternalInput")
    y_out = nc.dram_tensor("y", [S, DIM], bf16, kind="ExternalOutput")

    with tile.TileContext(nc) as tc, ExitStack() as top:
        const = top.enter_context(tc.tile_pool(name="const", bufs=1))
        pers = top.enter_context(tc.tile_pool(name="pers", bufs=1))
        qrp = top.enter_context(tc.tile_pool(name="qrp", bufs=1))
        glob = top.enter_context(tc.tile_pool(name="glob", bufs=1))
        p3s = top.enter_context(ExitStack())
        pP3 = p3s.enter_context(tc.tile_pool(name="pP3", bufs=3,
                                             space="PSUM"))

        sb_aux = const.tile([128, A_N], bf16, tag="aux", name="sb_aux")
        sb_b2 = const.tile([2, 128], f32r, tag="b2", name="sb_b2")
        sb_g = const.tile([8, 2], f32, tag="gains", name="sb_g")
        sb_wp = pers.tile([128, NC * 2 * DIM], fp8, tag="wp", name="sb_wp")

        sb_cos = sb_aux[:, A_COS:A_COS + S]
        sb_sin = sb_aux[:, A_SIN:A_SIN + S]
        sb_rt = sb_aux[:, A_RT:A_RT + 128]
        sb_id = sb_aux[:, A_ID:A_ID + 128]
        sb_mts = sb_aux[:, A_MTS:A_MTS + 128]
        sb_one = sb_aux[:, A_ONE:A_ONE + 1]

        def bsq(c):          # [128, 16] q sum-sq indicator for q chunk c
            o = A_BSQ + 16 * c
            return sb_aux[:, o:o + 16]

        qf = [None] * 8    # final scaled+roped qT chunks (bf16)
        kd = []            # kv head rows duplicated to both partition halves
        vsb = [None] * 8   # token-major v with ones column per kv head
        qr = {}            # roped, unscaled chunks

        # tiles that must survive into stage 2 (A = chunks 0-3, B = 4-7)
        scallA = glob.tile([8, S], f32r, tag="scallA", name="scallA")
        scallB = glob.tile([8, S], f32r, tag="scallB", name="scallB")
        kl_t = glob.tile([128, 32], f32, tag="klt", name="kl_t")
        skT = glob.tile([128, 32], f32, tag="skT", name="skT")
        sb_eps = glob.tile([128, 1], f32, tag="eps", name="sb_eps")
        s128 = glob.tile([128, 128], bf16, tag="s128", name="s128")
        ytf8 = glob.tile([128, NC * 2 * S], fp8, tag="ytf8", name="ytf8")
        nc.vector.memset(sb_eps, EPS)
        nc.vector.memset(s128, 1.0)   # unwritten rows must stay recip-safe

        rcp = top.enter_context(tc.tile_pool(name="rcp", bufs=2))

        # ---------------- stage 1: projections, rms-norm stats, rope -------
        with ExitStack() as s1:
            s1b = s1.enter_context(tc.tile_pool(name="s1b", bufs=1))
            w1 = s1.enter_context(tc.tile_pool(name="w1", bufs=1))
            sb_wqk = w1.tile([128, 10 * NC * 256], fp8, tag="wqk",
                             name="sb_wqk")
            sb_x = w1.tile([128, NC * 2 * S], fp8, tag="x", name="sb_x")
            sb_wv = w1.tile([128, NC * 512], fp8, tag="wv", name="sb_wv")
            xv = sb_x.rearrange("p (kc two s) -> p kc two s", two=2, s=S)
            wvv = sb_wv.rearrange("p (kc two m) -> p kc two m", two=2,
                                  m=256)

            def v_chunk(t):
                psv = pP3.tile([128, 256], f32, tag="pp", name=f"ps_v{t}")
                for kc in range(0, NC, 2):
                    nc.tensor.matmul(
                        psv,
                        lhsT=xv[:, kc:kc + 2, 1, t * 128:(t + 1) * 128],
                        rhs=wvv[:, kc:kc + 2, 0, :],
                        start=(kc == 0), stop=False, perf_mode=DR)
                for kc in range(NC):
                    nc.tensor.matmul(
                        psv,
                        lhsT=xv[:, kc, :, t * 128:(t + 1) * 128],
                        rhs=wvv[:, kc, :, :],
                        start=False, stop=(kc == NC - 1), perf_mode=DR)
                vt = pers.tile([128, 260], bf16, tag=f"v{t}", name=f"v{t}")
                vsb[t] = vt
                vt_g = vt.rearrange("p (g x) -> p g x", x=65)
                nc.vector.tensor_scalar_mul(
                    vt_g[:, :, 0:64],
                    psv.rearrange("p (g x) -> p g x", x=64), 1.0 / WSC)
                nc.vector.memset(vt_g[:, :, 64:65], 1.0)

            # warm the exp table (Copy/Exp share it; Sqrt swaps once later,
            # before the exp stream starts)
            dumm = s1b.tile([1, 4], f32, tag="dumm", name="dumm")
            dumo = s1b.tile([1, 4], f32, tag="dumo", name="dumo")
            nc.vector.memset(dumm, 1.0)
            nc.scalar.activation(dumo, dumm, Exp)

            # loads: k-row weights and x first so the k projection starts
            # ASAP; aux right after (rope tables needed by ~8us).
            xq = NC * 2 * S // 4
            nc.scalar.dma_start(out=sb_wqk[:, 8 * 2048:10 * 2048],
                                in_=inp["wqk"][:, 8 * 2048:10 * 2048])
            for i in range(4):
                nc.sync.dma_start(out=sb_x[:, i * xq:(i + 1) * xq],
                                  in_=inp["xf"][:, i * xq:(i + 1) * xq])
            nc.gpsimd.dma_start(out=sb_aux, in_=inp["aux"][:])
            for i in range(4):
                nc.scalar.dma_start(out=sb_wqk[:, i * 4096:(i + 1) * 4096],
                                    in_=inp["wqk"][:, i * 4096:(i + 1) * 4096])
            nc.gpsimd.dma_start(out=sb_b2, in_=inp["b2"][:])
            nc.gpsimd.dma_start(out=sb_g, in_=inp["gains"][:])
            nc.gpsimd.dma_start(out=sb_wv, in_=inp["wv"][:])
            nc.gpsimd.dma_start(out=sb_wp, in_=inp["wp"][:])

            wqkv = sb_wqk.rearrange("p (oc kc two m) -> p oc kc two m",
                                    oc=10, kc=NC, two=2)

            tmp = s1.enter_context(tc.tile_pool(name="tmp", bufs=2))
            psQ = s1.enter_context(tc.tile_pool(name="psQ", bufs=1,
                                                space="PSUM"))
            psK = s1.enter_context(tc.tile_pool(name="psK", bufs=1,
                                                space="PSUM"))
            ps_sqA = psQ.tile([8, S], f32, tag="sqA", name="ps_sqA")
            ps_sqB = psQ.tile([8, S], f32, tag="sqB", name="ps_sqB")
            ps_sk = psK.tile([128, 32], f32, tag="sk", name="ps_sk")

            def proj_mm(ps, oc, n):
                """[128,512] psum half n of output chunk oc via fp8 DoubleRow."""
                for kc in range(0, NC, 2):
                    nc.tensor.matmul(
                        ps,
                        lhsT=wqkv[:, oc, kc:kc + 2, 0, :],
                        rhs=xv[:, kc:kc + 2, 1, n * 512:(n + 1) * 512],
                        start=(kc == 0), stop=False, perf_mode=DR)
                for kc in range(NC):
                    nc.tensor.matmul(
                        ps,
                        lhsT=wqkv[:, oc, kc, :, :],
                        rhs=xv[:, kc, :, n * 512:(n + 1) * 512],
                        start=False, stop=(kc == NC - 1), perf_mode=DR)

            def rope(c, qsb):
                """rotate-half via +-1 permutation matmul + signed muls."""
                t1 = tmp.tile([128, S], bf16, tag="t1", name=f"t1_{c}")
                nc.vector.tensor_mul(t1, qsb, sb_cos)
                tag = f"qr{c}" if c < 8 else "qrk"
                qrc = qrp.tile([128, S], bf16, tag=tag, name=f"qr{c}",
                               bufs=1)
                for n in range(2):
                    pr = pP3.tile([128, 512], f32, tag="pp", name=f"pr{c}_{n}")
                    nc.tensor.matmul(pr, lhsT=sb_rt,
                                     rhs=qsb[:, n * 512:(n + 1) * 512],
                                     start=True, stop=True)
                    t2 = tmp.tile([128, 512], bf16, tag="t2", name=f"t2{c}_{n}")
                    nc.vector.tensor_mul(t2, pr,
                                         sb_sin[:, n * 512:(n + 1) * 512])
                    nc.vector.tensor_add(qrc[:, n * 512:(n + 1) * 512],
                                         t1[:, n * 512:(n + 1) * 512], t2)
                return qrc

            def qkv_chunk(c):
                qsb = tmp.tile([128, S], bf16, tag="qs", name=f"qsb{c}")
                for n in range(2):
                    ps = pP3.tile([128, 512], f32, tag="pp", name=f"pj{c}_{n}")
                    proj_mm(ps, c, n)
                    nc.scalar.activation(qsb[:, n * 512:(n + 1) * 512], ps,
                                         Copy, scale=1.0 / WSC)
                q2 = tmp.tile([128, S], bf16, tag="q2", name=f"q2_{c}")
                nc.vector.tensor_mul(q2, qsb, qsb)
                if c < 8:
                    sq_out = ps_sqA if c < 4 else ps_sqB
                    lh = bsq(c)[:, 0:8] if c < 4 else bsq(c)[:, 8:16]
                    for n in range(2):
                        nc.tensor.matmul(
                            sq_out[:, n * 512:(n + 1) * 512], lhsT=lh,
                            rhs=q2[:, n * 512:(n + 1) * 512],
                            start=(c % 4 == 0), stop=(c % 4 == 3))
                else:
                    for gg in range(2):
                        g = 2 * (c - 8) + gg
                        for m in range(NC):
                            nc.tensor.matmul(
                                ps_sk[:, g * 8 + m:g * 8 + m + 1],
                                lhsT=q2[gg * 64:(gg + 1) * 64,
                                        m * 128:(m + 1) * 128],
                                rhs=sb_one[gg * 64:(gg + 1) * 64, :],
                                start=True, stop=True)
                qrc = rope(c, qsb)
                qr[c] = qrc
                if c >= 8:
                    for gg in range(2):
                        g = 2 * (c - 8) + gg
                        kdg = pers.tile([128, S], bf16, tag=f"kd{g}",
                                        name=f"kd{g}")
                        kd.append(kdg)
                        srck = qrc[gg * 64:gg * 64 + 64, :]
                        eng = nc.sync if gg == 0 else nc.scalar
                        eng.dma_start(out=kdg[0:64, :], in_=srck)
                        eng.dma_start(out=kdg[64:128, :], in_=srck)

            f16 = mybir.dt.float16
            fins = [None] * 8
            t_sqA = s1b.tile([8, S], f32, tag="tsqA", name="t_sqA")
            t_sqB = s1b.tile([8, S], f32, tag="tsqB", name="t_sqB")
            t_invA = s1b.tile([8, S], f32, tag="tinvA", name="t_invA")
            t_invB = s1b.tile([8, S], f32, tag="tinvB", name="t_invB")

            def fin_dma(c):
                fin = rcp.tile([2, S], f32r, tag=f"fin{c}", name=f"fin{c}",
                               bufs=1)
                fins[c] = fin
                src = scallA if c < 4 else scallB
                cc = c % 4
                eng = (nc.sync, nc.scalar)[c % 2]
                eng.dma_start(out=fin, in_=src[2 * cc:2 * cc + 2, :])

            def finish(c):
                qfc = pers.tile([128, S], bf16, tag=f"qf{c}", name=f"qf{c}")
                qf[c] = qfc
                # psum broadcast drains through an ACT copy (f16) so the DVE
                # multiply is a cheap all-SBUF 2-byte op and the psum ring
                # never waits on DVE.
                pbf = rcp.tile([128, S], f16, tag="pbf", name=f"pbf{c}",
                               bufs=1)
                for n in range(2):
                    pb = pP3.tile([128, 512], f32, tag="pp",
                                  name=f"fb{c}_{n}")
                    nc.tensor.matmul(
                        pb, lhsT=sb_b2,
                        rhs=fins[c][:, n * 512:(n + 1) * 512],
                        start=True, stop=True)
                    nc.scalar.activation(pbf[:, n * 512:(n + 1) * 512], pb,
                                         Copy)
                nc.vector.tensor_mul(qfc, qr[c], pbf)

            qkv_chunk(8)
            qkv_chunk(9)
            # k scale chain immediately (skT only feeds the exp scale)
            nc.scalar.activation(kl_t, ps_sk, Sqrt, scale=1.0 / HD,
                                 bias=sb_eps)
            nc.vector.reciprocal_approx_fast(skT, kl_t)
            for c in range(4):
                qkv_chunk(c)
                if c >= 1:
                    v_chunk(c - 1)
            # chain A: chunks 0-3 scales ready while 4-7 still project
            nc.scalar.activation(t_sqA, ps_sqA, Sqrt, scale=1.0 / HD,
                                 bias=sb_eps[0:8, :])
            nc.vector.reciprocal_approx_fast(t_invA, t_sqA)
            nc.vector.tensor_scalar_mul(scallA, t_invA, sb_g[:, 0:1])
            for c in range(4):
                fin_dma(c)
            for c in range(4, 8):
                qkv_chunk(c)
                finish(c - 4)
                v_chunk(c - 1)
            v_chunk(7)
            nc.scalar.activation(t_sqB, ps_sqB, Sqrt, scale=1.0 / HD,
                                 bias=sb_eps[0:8, :])
            nc.vector.reciprocal_approx_fast(t_invB, t_sqB)
            nc.vector.tensor_scalar_mul(scallB, t_invB, sb_g[:, 1:2])
            for c in range(4, 8):
                fin_dma(c)
            for c in range(4, 8):
                finish(c)
        p3s.close()

        # ---------------- stage 2: attention --------------------------------
        with ExitStack() as s23:
            s2 = s23.enter_context(ExitStack())
            pP = s2.enter_context(tc.tile_pool(name="pP", bufs=3))
            stg = s2.enter_context(tc.tile_pool(name="stg", bufs=3))
            ytp = s2.enter_context(tc.tile_pool(name="ytp", bufs=2))
            psS = s2.enter_context(tc.tile_pool(name="psS", bufs=3,
                                                space="PSUM"))
            psY = s2.enter_context(tc.tile_pool(name="psY", bufs=2,
                                                space="PSUM"))

            yv8 = ytf8.rearrange("p (dc two s) -> p dc two s", two=2, s=S)

            def attn_scores(hp):
                h0, h1 = 2 * hp, 2 * hp + 1
                g = h0 // 4
                Ppair = {h0: [], h1: []}
                for m in range(NC):
                    w = S - 128 * m
                    sk_col = skT[:, g * 8 + m:g * 8 + m + 1]
                    merged = w <= 512
                    if merged:
                        psm = psS.tile([128, S], f32, tag="sc",
                                       name=f"sc{hp}_{m}")
                        pss = {h0: psm[:, 0:w], h1: psm[:, 512:512 + w]}
                    else:
                        pss = {h: psS.tile([128, S], f32, tag="sc",
                                           name=f"sc{h}_{m}")[:, 0:w]
                               for h in (h0, h1)}
                    for h in (h0, h1):
                        # causal mask for the diagonal block: M^T @ I adds
                        # -60 above the diagonal, so exp() zeroes it.
                        nc.tensor.matmul(
                            pss[h][:, 0:128], lhsT=sb_mts, rhs=sb_id,
                            start=True, stop=False, skip_group_check=True)
                    for n0 in range(0, w, 512):
                        nw = min(512, w - n0)
                        for h in (h0, h1):
                            b = (h % 2) * 64
                            if n0 == 0:
                                nc.tensor.matmul(
                                    pss[h][:, 0:128],
                                    lhsT=kd[g][b:b + 64, m * 128:(m + 1) * 128],
                                    rhs=qf[hp][b:b + 64,
                                               128 * m:128 * m + 128],
                                    start=False, stop=True,
                                    skip_group_check=True)
                                if nw > 128:
                                    nc.tensor.matmul(
                                        pss[h][:, 128:nw],
                                        lhsT=kd[g][b:b + 64,
                                                   m * 128:(m + 1) * 128],
                                        rhs=qf[hp][b:b + 64,
                                                   128 * m + 128:128 * m + nw],
                                        start=True, stop=True,
                                        skip_group_check=True)
                            else:
                                nc.tensor.matmul(
                                    pss[h][:, n0:n0 + nw],
                                    lhsT=kd[g][b:b + 64, m * 128:(m + 1) * 128],
                                    rhs=qf[hp][b:b + 64,
                                               128 * m + n0:128 * m + n0 + nw],
                                    start=True, stop=True,
                                    skip_group_check=True)
                    if merged:
                        pmm = pP.tile([128, 2, 512], bf16, tag=f"P{m}",
                                      name=f"P{hp}_{m}", bufs=2)
                        nc.scalar.activation(
                            pmm[:, :, 0:w],
                            psm.rearrange("p (t x) -> p t x", x=512)[:, :, 0:w],
                            Exp, scale=sk_col)
                        Ppair[h0].append(pmm[:, 0, :][:, 0:w])
                        Ppair[h1].append(pmm[:, 1, :][:, 0:w])
                    else:
                        for h in (h0, h1):
                            # 2 allocs per pair -> ring 4 = two pairs deep
                            pm = pP.tile([128, S], bf16, tag=f"Pb{m}",
                                         name=f"P{h}_{m}", bufs=4)
                            nc.scalar.activation(pm[:, 0:w], pss[h], Exp,
                                                 scale=sk_col)
                            Ppair[h].append(pm[:, 0:w])
                return Ppair

            def attn_pv(hp, Ppair):
                h0, h1 = 2 * hp, 2 * hp + 1
                g = h0 // 4
                ytc = ytp.tile([128, S], bf16, tag="ytp", name=f"ytp{hp}")
                for h in (h0, h1):
                    P = Ppair[h]
                    yh = stg.tile([65, S], bf16, tag="yh", name=f"yh{h}")
                    for j in range(2):
                        py = psY.tile([65, 512], f32, tag="y",
                                      name=f"py{h}_{j}")
                        for m in range(4 * j + 4):
                            if m <= 4 * j:
                                o0, c0, nw = 0, 512 * j - 128 * m, 512
                            else:
                                o0 = 128 * m - 512 * j
                                c0, nw = 0, 512 - o0
                            nc.tensor.matmul(
                                py[:, o0:o0 + nw],
                                lhsT=vsb[m][:, 65 * g:65 * g + 65],
                                rhs=P[m][:, c0:c0 + nw],
                                start=(m == 0), stop=(m == 4 * j + 3),
                                skip_group_check=True)
                        nc.vector.tensor_copy(yh[:, j * 512:(j + 1) * 512], py)
                    # last pair: exp stream is over, use the fast HWDGE queue
                    deng = nc.gpsimd if hp < 7 else nc.scalar
                    deng.dma_start(out=ytc[(h % 2) * 64:(h % 2) * 64 + 64, :],
                                   in_=yh[0:64, :])
                    r0 = 64 * (h % 2) + 8 * (h // 2)
                    deng.dma_start(out=s128[r0:r0 + 8, :], in_=yh[64:65, :])
                return ytc

            def normA(hp):
                """recip of denominators + row flatten (no PE work)."""
                b32 = 32 * (hp // 4)
                o8 = 8 * (hp % 4)
                rc = rcp.tile([64, 128], f32, tag="rc", name=f"rc{hp}")
                nc.vector.tensor_copy(rc[0:32, :], s128[b32:b32 + 32, :])
                nc.vector.tensor_copy(rc[32:64, :],
                                      s128[64 + b32:64 + b32 + 32, :])
                rr = rcp.tile([64, 128], f32, tag="rr", name=f"rr{hp}")
                nc.vector.reciprocal_approx_fast(rr, rc)
                rnt = rcp.tile([2, S], f32r, tag="rnt", name=f"rnt{hp}")
                # keep these OFF the scalar queue: a data-dependent DMA there
                # stalls the Activation sequencer mid-exp-stream.
                nc.sync.dma_start(
                    out=rnt[0:1, :], in_=rr[o8:o8 + 8, :].bitcast(f32r))
                nc.gpsimd.dma_start(
                    out=rnt[1:2, :],
                    in_=rr[32 + o8:32 + o8 + 8, :].bitcast(f32r))
                return rnt

            def normB(hp, ytc, rnt):
                """broadcast, apply, hi/lo fp8 split of y^T."""
                for n in range(2):
                    pb = psY.tile([128, 512], f32, tag="y",
                                  name=f"nb{hp}_{n}")
                    nc.tensor.matmul(
                        pb, lhsT=sb_b2,
                        rhs=rnt[:, n * 512:(n + 1) * 512],
                        start=True, stop=True)
                    nc.vector.tensor_mul(ytc[:, n * 512:(n + 1) * 512],
                                         ytc[:, n * 512:(n + 1) * 512], pb)
                # hi/lo split: per dc layout [lo | hi]
                nc.vector.tensor_copy(yv8[:, hp, 1, :], ytc)
                nc.vector.tensor_sub(yv8[:, hp, 0, :], ytc, yv8[:, hp, 1, :])

            # scores run two pairs ahead of PV so the exp stream never waits
            # on PV.  normA(hp) (recip + rnt DMA launch, no PE) fires right
            # after PV(hp); normB(hp) (pb matmul + apply + split) runs one
            # iteration later when its rnt has landed — neither the PE nor
            # the DVE queue ever blocks on a norm chain in flight.
            P = {0: attn_scores(0), 1: attn_scores(1)}
            prev = None
            rnts = {}
            for hp in range(8):
                if hp + 2 < 8:
                    P[hp + 2] = attn_scores(hp + 2)
                ytc = attn_pv(hp, P.pop(hp))
                rnts[hp] = normA(hp)
                if prev is not None:
                    normB(hp - 1, prev, rnts.pop(hp - 1))
                prev = ytc
            for i in range(20):
                wt = psS.tile([128, S], f32, tag="sc", name=f"warm{i}")
                nc.tensor.matmul(wt[:, 0:512], lhsT=sb_id,
                                 rhs=sb_aux[:, 0:512], start=True, stop=True)
            normB(7, prev, rnts.pop(7))

            s2.close()

            # ---------------- stage 3: output projection (fp8) --------------
            with ExitStack() as s3:
                psO = s3.enter_context(tc.tile_pool(name="psO", bufs=4,
                                                    space="PSUM"))
                osb = s3.enter_context(tc.tile_pool(name="osb", bufs=1)).tile(
                    [128, NC * DIM], bf16, tag="osb", name="osb_all")
                wpv = sb_wp.rearrange("p (dc two d) -> p dc two d",
                                      two=2, d=DIM)
                yv = y_out.rearrange("(t p) d -> p t d", p=128)
                ov = osb.rearrange("p (t d) -> p t d", d=DIM)
                def opA(po, t, n):
                    # contributions from pairs 0..5 (normalized long ago)
                    for dc in range(0, 6, 2):
                        nc.tensor.matmul(
                            po,
                            lhsT=yv8[:, dc:dc + 2, 1, t * 128:(t + 1) * 128],
                            rhs=wpv[:, dc:dc + 2, 0, n * 512:(n + 1) * 512],
                            start=(dc == 0), stop=False, perf_mode=DR)
                    for dc in range(6):
                        nc.tensor.matmul(
                            po,
                            lhsT=yv8[:, dc, :, t * 128:(t + 1) * 128],
                            rhs=wpv[:, dc, :, n * 512:(n + 1) * 512],
                            start=False, stop=False, perf_mode=DR)

                def opB(po, t, n):
                    nc.tensor.matmul(
                        po, lhsT=yv8[:, 6:8, 1, t * 128:(t + 1) * 128],
                        rhs=wpv[:, 6:8, 0, n * 512:(n + 1) * 512],
                        start=False, stop=False, perf_mode=DR)
                    for dc in (6, 7):
                        nc.tensor.matmul(
                            po,
                            lhsT=yv8[:, dc, :, t * 128:(t + 1) * 128],
                            rhs=wpv[:, dc, :, n * 512:(n + 1) * 512],
                            start=False, stop=(dc == 7), perf_mode=DR)
                    nc.scalar.activation(
                        osb[:, t * DIM + n * 512:t * DIM + (n + 1) * 512],
                        po, Copy, scale=1.0 / WSC)
                    if n == 1:
                        eng = nc.sync if t % 2 == 0 else nc.scalar
                        eng.dma_start(out=yv[:, t:t + 1, :],
                                      in_=ov[:, t:t + 1, :])

                groups = [(t, n) for t in range(NC) for n in range(2)]
                live = []
                for t, n in groups:
                    po = psO.tile([128, 512], f32, tag="o",
                                  name=f"ps_o{t}_{n}")
                    opA(po, t, n)
                    live.append((po, t, n))
                    if len(live) == 4:
                        opB(*live.pop(0))
                for g in live:
                    opB(*g)
    nc.compile()
    return nc


def _split_f8(a):
    hi = a.astype(F8)
    lo = (a - hi.astype(np.float32)).astype(F8)
    return hi, lo


def _host_prep(x, Wq, Wk, Wv, Wproj, q_gain, q_scale, k_scale,
               rotary_cos, rotary_sin):
    # ---- fp8 weights: q rows then k rows, out-chunk major --------------
    wqk = np.concatenate([Wq, Wk], axis=0).astype(np.float32) * WSC
    w4 = wqk.reshape(10, 128, NC, 128)        # [oc, m, kc, p]
    hi, lo = _split_f8(w4)
    wqk_f8 = np.zeros((128, 10, NC, 2, 128), dtype=F8)
    wqk_f8[:, :, :, 0, :] = hi.transpose(3, 0, 2, 1)
    wqk_f8[:, :, :, 1, :] = lo.transpose(3, 0, 2, 1)
    wqk_f8 = np.ascontiguousarray(wqk_f8.reshape(128, 10 * NC * 256))

    wvm = (Wv.astype(np.float32) * WSC).reshape(256, NC, 128)  # [m, kc, p]
    hi, lo = _split_f8(wvm)
    wv_f8 = np.zeros((128, NC, 2, 256), dtype=F8)
    wv_f8[:, :, 0, :] = hi.transpose(2, 1, 0)
    wv_f8[:, :, 1, :] = lo.transpose(2, 1, 0)
    wv_f8 = np.ascontiguousarray(wv_f8.reshape(128, NC * 512))

    # wp lhs rows are y dims (dc chunks), rhs cols are output dims
    wpm = (Wproj.T.astype(np.float32) * WSC).reshape(NC, 128, DIM)  # [dc,p,d]
    hi, lo = _split_f8(wpm)
    wp_f8 = np.zeros((128, NC, 2, DIM), dtype=F8)
    wp_f8[:, :, 0, :] = hi.transpose(1, 0, 2)
    wp_f8[:, :, 1, :] = lo.transpose(1, 0, 2)
    wp_f8 = np.ascontiguousarray(wp_f8.reshape(128, NC * 2 * DIM))

    shared = {"wqk": wqk_f8, "wv": wv_f8, "wp": wp_f8}

    aux = np.zeros((128, A_N), dtype=np.float32)
    cos = np.asarray(rotary_cos, np.float32).reshape(B * S, HD // 2)[:S].T
    sin = np.asarray(rotary_sin, np.float32).reshape(B * S, HD // 2)[:S].T
    aux[:, A_COS:A_COS + S] = np.tile(cos, (4, 1))
    aux[:, A_SIN:A_SIN + S] = np.tile(sin, (4, 1))
    # rotate-half permutation (lhsT = R.T), exact in bf16
    R = np.zeros((128, 128), dtype=np.float32)
    for i in range(128):
        if i % 64 < 32:
            R[i, i + 32] = 1.0
        else:
            R[i, i - 32] = -1.0
    aux[:, A_RT:A_RT + 128] = R.T
    # q sum-sq indicators: head h -> row h (plain)
    for c in range(8):
        for r in range(128):
            h = 2 * c + r // 64
            aux[r, A_BSQ + 16 * c + h] = 1.0
    # identity + causal -60 seed for diagonal blocks of scores^T
    ar = np.arange(128)
    aux[:, A_ID:A_ID + 128] = np.eye(128, dtype=np.float32)
    aux[:, A_MTS:A_MTS + 128] = -60.0 * (ar[None, :] < ar[:, None]).T
    aux[:, A_ONE:A_ONE + 1] = 1.0
    shared["aux"] = aux.astype(BF)

    b2 = np.zeros((2, 128), dtype=np.float32)
    b2[0, 0:64] = 1.0
    b2[1, 64:128] = 1.0
    shared["b2"] = b2

    # q gains: col 0 = heads 0-7 (row h), col 1 = heads 8-15 (row h-8);
    # folds q_scale and HD^-0.5
    gg = np.zeros((8, 2), dtype=np.float32)
    qg = np.asarray(q_gain, np.float32) * float(q_scale) * (HD ** -0.5)
    for h in range(H):
        gg[h % 8, h // 8] = qg[h]
    shared["gains"] = gg
    assert abs(float(k_scale) - 1.0) < 1e-6, "k_scale fold not implemented"

    per_core = []
    x = np.asarray(x, np.float32)
    for c in range(B):
        xd = x[c * S:(c + 1) * S]                     # [1024 t, 1024 d]
        xT = xd.T.reshape(NC, 128, S)                 # [kc, p, t]
        hi, lo = _split_f8(xT)
        xf = np.zeros((128, NC, 2, S), dtype=F8)
        xf[:, :, 0, :] = lo.transpose(1, 0, 2)
        xf[:, :, 1, :] = hi.transpose(1, 0, 2)
        per_core.append({"xf": np.ascontiguousarray(
            xf.reshape(128, NC * 2 * S))})
    return shared, per_core


def kernel(x, Wq, Wk, Wv, Wproj, q_gain, q_scale, k_scale,
           rotary_cos, rotary_sin, cu_seqlens=None, max_doc_len=None,
           **_ignored):
    x = np.asarray(x, np.float32)
    assert x.shape == (B * S, DIM), x.shape
    if "nc" not in _CACHE:
        _CACHE["nc"] = _build()
    nc = _CACHE["nc"]
    shared, per_core = _host_prep(
        np.asarray(x, np.float32), np.asarray(Wq, np.float32),
        np.asarray(Wk, np.float32), np.asarray(Wv, np.float32),
        np.asarray(Wproj, np.float32), np.asarray(q_gain, np.float32),
        np.asarray(q_scale, np.float32), np.asarray(k_scale, np.float32),
        np.asarray(rotary_cos, np.float32), np.asarray(rotary_sin, np.float32))
    in_maps = [{**shared, **pc} for pc in per_core]
    res = bass_utils.run_bass_kernel_spmd(
        nc, in_maps, core_ids=list(range(B)),
        trace=bool(int(os.environ.get("KERNEL_TRACE", "0"))))
    _CACHE["last_results"] = res
    out = np.concatenate(
        [np.asarray(res.results[c]["y"]).astype(np.float32) for c in range(B)],
        axis=0)
    return out


# revision 71
# speedup vs baseline: 1.0078x; 1.0078x over previous
"""Trainium2 Bass kernel for causal GQA self-attention (8 docs x 1024 tokens,
dim 1024, 16 q heads / 4 kv heads, head_dim 64, RMS-normed+RoPE q/k).

Sharding: data-parallel over docs — core c computes doc c end to end.

v3 design notes (cost-model driven):
  - QKV/V projections and the output projection run in fp8-e4m3 DoubleRow
    mode (0.5 cycles/row): operands are split into hi+lo e4m3 parts (weights
    pre-scaled by 32 so the lo residual stays out of the subnormal range);
    slot-paired products hi*hi + (hi*lo + lo*hi) give better-than-bf16
    accuracy at 0.75x the bf16 PE cost.  Dequant rides psum->sbuf copies.
  - psum->sbuf copies of q/k (and the final output) ride the Activation
    engine, which is otherwise idle outside the exp stream.
  - k is never scaled on chip: s_k = rsqrt(mean k^2) is folded into the
    exp() activation's per-partition scale operand (scores^T layout has tk
    on partitions); k sum-squares are computed directly in column form by
    tiny ones-vector matmuls.  No causal-mask PE matmuls: the diagonal
    128x128 block of each P tile is multiplied by a 0/1 mask on DVE post-exp.
  - the attention phase is Activation-bound (exp stream ~80us), so all
    other PE work that can move there does: v projections, q-scale
    finishes, per-pair normalization, and the hi/lo split of y for the
    fp8 output projection are interleaved between head pairs.
  - one shared [128,512] psum ring (3 banks) serves projection halves,
    rope, v, scale broadcasts and PV; scores use a 2-deep [128,1024] ring.
"""

import os
import sys

sys.path.insert(0, "/opt/trn_rl_repo")

import numpy as np
import ml_dtypes

import concourse.bass as bass
import concourse.bacc as bacc
import concourse.mybir as mybir
import concourse.tile as tile
from concourse import bass_utils
from contextlib import ExitStack

f32 = mybir.dt.float32
f32r = mybir.dt.float32r
bf16 = mybir.dt.bfloat16
fp8 = mybir.dt.float8e4
BF = ml_dtypes.bfloat16
F8 = ml_dtypes.float8_e4m3fn
DR = mybir.MatmulPerfMode.DoubleRow

DIM = 1024
H = 16
HKV = 4
HD = 64
B = 8
S = 1024
NC = 8          # d chunks of 128
WSC = 32.0      # fp8 weight pre-scale
EPS = float(np.finfo(np.float32).eps)
Exp = mybir.ActivationFunctionType.Exp
Sqrt = mybir.ActivationFunctionType.Sqrt
Copy = mybir.ActivationFunctionType.Copy

# aux blob column offsets (all [128, n] bf16)
A_COS, A_SIN, A_RT, A_BSQ = 0, 1024, 2048, 2176
A_ID, A_MTS, A_ONE = 2304, 2432, 2560
A_N = 2561
_CACHE = {}


def _build():
    nc = bacc.Bacc("TRN2")
    inp = {}
    for name, shape, dt in [
        ("xf", [128, NC * 2 * S], fp8),         # per kc: [lo S | hi S]
        ("wqk", [128, 10 * NC * 256], fp8),     # per oc: kc-major [hi | lo]
        ("wv", [128, NC * 512], fp8),           # per kc: [hi 256 | lo 256]
        ("wp", [128, NC * 2 * DIM], fp8),       # per dc: kc-major [hi | lo]
        ("aux", [128, A_N], bf16),
        ("b2", [2, 128], f32r),
        ("gains", [16, 1], f32),
    ]:
        inp[name] = nc.dram_tensor(name, shape, dt, kind="ExternalInput")
    y_out = nc.dram_tensor("y", [S, DIM], bf16, kind="ExternalOutput")

    with tile.TileContext(nc) as tc, ExitStack() as top:
        const = top.enter_context(tc.tile_pool(name="const", bufs=1))
        pers = top.enter_context(tc.tile_pool(name="pers", bufs=1))
        qrp = top.enter_context(tc.tile_pool(name="qrp", bufs=1))
        glob = top.enter_context(tc.tile_pool(name="glob", bufs=1))
        p3s = top.enter_context(ExitStack())
        pP3 = p3s.enter_context(tc.tile_pool(name="pP3", bufs=3,
                                             space="PSUM"))

        sb_aux = const.tile([128, A_N], bf16, tag="aux", name="sb_aux")
        sb_b2 = const.tile([2, 128], f32r, tag="b2", name="sb_b2")
        sb_g = const.tile([16, 1], f32, tag="gains", name="sb_g")
        sb_wp = pers.tile([128, NC * 2 * DIM], fp8, tag="wp", name="sb_wp")

        sb_cos = sb_aux[:, A_COS:A_COS + S]
        sb_sin = sb_aux[:, A_SIN:A_SIN + S]
        sb_rt = sb_aux[:, A_RT:A_RT + 128]
        sb_id = sb_aux[:, A_ID:A_ID + 128]
        sb_mts = sb_aux[:, A_MTS:A_MTS + 128]
        sb_one = sb_aux[:, A_ONE:A_ONE + 1]

        def bsq(c):          # [128, 16] q sum-sq indicator for q chunk c
            o = A_BSQ + 16 * c
            return sb_aux[:, o:o + 16]

        qf = [None] * 8    # final scaled+roped qT chunks (bf16)
        kd = []            # kv head rows duplicated to both partition halves
        vsb = [None] * 8   # token-major v with ones column per kv head
        qr = {}            # roped, unscaled chunks

        # tiles that must survive into stage 2
        t_sq = glob.tile([16, S], f32, tag="tsq", name="t_sq")
        t_inv = glob.tile([16, S], f32, tag="tinv", name="t_inv")
        scall = glob.tile([16, S], f32r, tag="scall", name="scall")
        kl_t = glob.tile([128, 32], f32, tag="klt", name="kl_t")
        skT = glob.tile([128, 32], f32, tag="skT", name="skT")
        sb_eps = glob.tile([128, 1], f32, tag="eps", name="sb_eps")
        s128 = glob.tile([128, 128], bf16, tag="s128", name="s128")
        ytf8 = glob.tile([128, NC * 2 * S], fp8, tag="ytf8", name="ytf8")
        nc.vector.memset(sb_eps, EPS)
        nc.vector.memset(s128, 1.0)   # unwritten rows must stay recip-safe

        # ---------------- stage 1: projections, rms-norm stats, rope -------
        with ExitStack() as s1:
            s1b = s1.enter_context(tc.tile_pool(name="s1b", bufs=1))
            w1 = s1.enter_context(tc.tile_pool(name="w1", bufs=1))
            sb_wqk = w1.tile([128, 10 * NC * 256], fp8, tag="wqk",
                             name="sb_wqk")
            sb_x = w1.tile([128, NC * 2 * S], fp8, tag="x", name="sb_x")
            sb_wv = w1.tile([128, NC * 512], fp8, tag="wv", name="sb_wv")
            xv = sb_x.rearrange("p (kc two s) -> p kc two s", two=2, s=S)
            wvv = sb_wv.rearrange("p (kc two m) -> p kc two m", two=2,
                                  m=256)

            def v_chunk(t):
                psv = pP3.tile([128, 256], f32, tag="pp", name=f"ps_v{t}")
                for kc in range(0, NC, 2):
                    nc.tensor.matmul(
                        psv,
                        lhsT=xv[:, kc:kc + 2, 1, t * 128:(t + 1) * 128],
                        rhs=wvv[:, kc:kc + 2, 0, :],
                        start=(kc == 0), stop=False, perf_mode=DR)
                for kc in range(NC):
                    nc.tensor.matmul(
                        psv,
                        lhsT=xv[:, kc, :, t * 128:(t + 1) * 128],
                        rhs=wvv[:, kc, :, :],
                        start=False, stop=(kc == NC - 1), perf_mode=DR)
                vt = pers.tile([128, 260], bf16, tag=f"v{t}", name=f"v{t}")
                vsb[t] = vt
                vt_g = vt.rearrange("p (g x) -> p g x", x=65)
                nc.vector.tensor_scalar_mul(
                    vt_g[:, :, 0:64],
                    psv.rearrange("p (g x) -> p g x", x=64), 1.0 / WSC)
                nc.vector.memset(vt_g[:, :, 64:65], 1.0)

            # warm the exp table (Copy/Exp share it; Sqrt swaps once later,
            # before the exp stream starts)
            dumm = s1b.tile([1, 4], f32, tag="dumm", name="dumm")
            dumo = s1b.tile([1, 4], f32, tag="dumo", name="dumo")
            nc.vector.memset(dumm, 1.0)
            nc.scalar.activation(dumo, dumm, Exp)

            # loads: k-row weights and x first so the k projection starts
            # ASAP; aux right after (rope tables needed by ~8us).
            xq = NC * 2 * S // 4
            nc.scalar.dma_start(out=sb_wqk[:, 8 * 2048:10 * 2048],
                                in_=inp["wqk"][:, 8 * 2048:10 * 2048])
            for i in range(4):
                nc.sync.dma_start(out=sb_x[:, i * xq:(i + 1) * xq],
                                  in_=inp["xf"][:, i * xq:(i + 1) * xq])
            nc.gpsimd.dma_start(out=sb_aux, in_=inp["aux"][:])
            for i in range(4):
                nc.scalar.dma_start(out=sb_wqk[:, i * 4096:(i + 1) * 4096],
                                    in_=inp["wqk"][:, i * 4096:(i + 1) * 4096])
            nc.gpsimd.dma_start(out=sb_b2, in_=inp["b2"][:])
            nc.gpsimd.dma_start(out=sb_g, in_=inp["gains"][:])
            nc.gpsimd.dma_start(out=sb_wv, in_=inp["wv"][:])
            nc.gpsimd.dma_start(out=sb_wp, in_=inp["wp"][:])

            wqkv = sb_wqk.rearrange("p (oc kc two m) -> p oc kc two m",
                                    oc=10, kc=NC, two=2)

            tmp = s1.enter_context(tc.tile_pool(name="tmp", bufs=2))
            psQ = s1.enter_context(tc.tile_pool(name="psQ", bufs=1,
                                                space="PSUM"))
            psK = s1.enter_context(tc.tile_pool(name="psK", bufs=1,
                                                space="PSUM"))
            ps_sq = psQ.tile([16, S], f32, tag="sq", name="ps_sq")
            ps_sk = psK.tile([128, 32], f32, tag="sk", name="ps_sk")

            def proj_mm(ps, oc, n):
                """[128,512] psum half n of output chunk oc via fp8 DoubleRow."""
                for kc in range(0, NC, 2):
                    nc.tensor.matmul(
                        ps,
                        lhsT=wqkv[:, oc, kc:kc + 2, 0, :],
                        rhs=xv[:, kc:kc + 2, 1, n * 512:(n + 1) * 512],
                        start=(kc == 0), stop=False, perf_mode=DR)
                for kc in range(NC):
                    nc.tensor.matmul(
                        ps,
                        lhsT=wqkv[:, oc, kc, :, :],
                        rhs=xv[:, kc, :, n * 512:(n + 1) * 512],
                        start=False, stop=(kc == NC - 1), perf_mode=DR)

            def rope(c, qsb):
                """rotate-half via +-1 permutation matmul + signed muls."""
                t1 = tmp.tile([128, S], bf16, tag="t1", name=f"t1_{c}")
                nc.vector.tensor_mul(t1, qsb, sb_cos)
                tag = f"qr{c}" if c < 8 else "qrk"
                qrc = qrp.tile([128, S], bf16, tag=tag, name=f"qr{c}",
                               bufs=1)
                for n in range(2):
                    pr = pP3.tile([128, 512], f32, tag="pp", name=f"pr{c}_{n}")
                    nc.tensor.matmul(pr, lhsT=sb_rt,
                                     rhs=qsb[:, n * 512:(n + 1) * 512],
                                     start=True, stop=True)
                    t2 = tmp.tile([128, 512], bf16, tag="t2", name=f"t2{c}_{n}")
                    nc.vector.tensor_mul(t2, pr,
                                         sb_sin[:, n * 512:(n + 1) * 512])
                    nc.vector.tensor_add(qrc[:, n * 512:(n + 1) * 512],
                                         t1[:, n * 512:(n + 1) * 512], t2)
                return qrc

            def qkv_chunk(c):
                qsb = tmp.tile([128, S], bf16, tag="qs", name=f"qsb{c}")
                for n in range(2):
                    ps = pP3.tile([128, 512], f32, tag="pp", name=f"pj{c}_{n}")
                    proj_mm(ps, c, n)
                    nc.scalar.activation(qsb[:, n * 512:(n + 1) * 512], ps,
                                         Copy, scale=1.0 / WSC)
                q2 = tmp.tile([128, S], bf16, tag="q2", name=f"q2_{c}")
                nc.vector.tensor_mul(q2, qsb, qsb)
                if c < 8:
                    for n in range(2):
                        nc.tensor.matmul(
                            ps_sq[:, n * 512:(n + 1) * 512], lhsT=bsq(c),
                            rhs=q2[:, n * 512:(n + 1) * 512],
                            start=(c == 0), stop=(c == 7))
                else:
                    for gg in range(2):
                        g = 2 * (c - 8) + gg
                        for m in range(NC):
                            nc.tensor.matmul(
                                ps_sk[:, g * 8 + m:g * 8 + m + 1],
                                lhsT=q2[gg * 64:(gg + 1) * 64,
                                        m * 128:(m + 1) * 128],
                                rhs=sb_one[gg * 64:(gg + 1) * 64, :],
                                start=True, stop=True)
                qrc = rope(c, qsb)
                qr[c] = qrc
                if c >= 8:
                    for gg in range(2):
                        g = 2 * (c - 8) + gg
                        kdg = pers.tile([128, S], bf16, tag=f"kd{g}",
                                        name=f"kd{g}")
                        kd.append(kdg)
                        srck = qrc[gg * 64:gg * 64 + 64, :]
                        eng = nc.sync if gg == 0 else nc.scalar
                        eng.dma_start(out=kdg[0:64, :], in_=srck)
                        eng.dma_start(out=kdg[64:128, :], in_=srck)

            qkv_chunk(8)
            qkv_chunk(9)
            # k scale chain immediately (skT only feeds the exp scale)
            nc.scalar.activation(kl_t, ps_sk, Sqrt, scale=1.0 / HD,
                                 bias=sb_eps)
            nc.vector.reciprocal_approx_fast(skT, kl_t)
            for c in range(8):
                qkv_chunk(c)
                if c >= 1:
                    v_chunk(c - 1)
            v_chunk(7)

            # q scale chain: Sqrt, then DVE reciprocal * gains.
            nc.scalar.activation(t_sq, ps_sq, Sqrt, scale=1.0 / HD,
                                 bias=sb_eps[0:16, :])
            nc.vector.reciprocal_approx_fast(t_inv, t_sq)
            nc.vector.tensor_scalar_mul(scall, t_inv, sb_g)

        # ---------------- stage 1b: finish q scales (shared psum ring) ------
        rcp = top.enter_context(tc.tile_pool(name="rcp", bufs=2))

        # all fin rows leave on DMA queues first; pb matmuls then stream
        # without per-chunk DMA latency in the PE path.
        fins = []
        for c in range(8):
            fin = rcp.tile([2, S], f32r, tag=f"fin{c}", name=f"fin{c}",
                           bufs=1)
            fins.append(fin)
            eng = (nc.sync, nc.scalar)[c % 2]
            eng.dma_start(out=fin, in_=scall[2 * c:2 * c + 2, :])

        f16 = mybir.dt.float16

        def finish(c):
            qfc = pers.tile([128, S], bf16, tag=f"qf{c}", name=f"qf{c}")
            qf[c] = qfc
            # psum broadcast drains through an ACT copy (f16) so the DVE
            # multiply is a cheap all-SBUF 2-byte op and the psum ring never
            # waits on DVE.
            pbf = rcp.tile([128, S], f16, tag="pbf", name=f"pbf{c}", bufs=2)
            for n in range(2):
                pb = pP3.tile([128, 512], f32, tag="pp", name=f"fb{c}_{n}")
                nc.tensor.matmul(
                    pb, lhsT=sb_b2,
                    rhs=fins[c][:, n * 512:(n + 1) * 512],
                    start=True, stop=True)
                nc.scalar.activation(pbf[:, n * 512:(n + 1) * 512], pb, Copy)
            nc.vector.tensor_mul(qfc, qr[c], pbf)

        for c in range(8):
            finish(c)
        p3s.close()

        # ---------------- stage 2: attention --------------------------------
        with ExitStack() as s23:
            s2 = s23.enter_context(ExitStack())
            pP = s2.enter_context(tc.tile_pool(name="pP", bufs=3))
            stg = s2.enter_context(tc.tile_pool(name="stg", bufs=3))
            ytp = s2.enter_context(tc.tile_pool(name="ytp", bufs=2))
            psS = s2.enter_context(tc.tile_pool(name="psS", bufs=3,
                                                space="PSUM"))
            psY = s2.enter_context(tc.tile_pool(name="psY", bufs=2,
                                                space="PSUM"))

            yv8 = ytf8.rearrange("p (dc two s) -> p dc two s", two=2, s=S)

            def attn_scores(hp):
                h0, h1 = 2 * hp, 2 * hp + 1
                g = h0 // 4
                Ppair = {h0: [], h1: []}
                for m in range(NC):
                    w = S - 128 * m
                    sk_col = skT[:, g * 8 + m:g * 8 + m + 1]
                    merged = w <= 512
                    if merged:
                        psm = psS.tile([128, S], f32, tag="sc",
                                       name=f"sc{hp}_{m}")
                        pss = {h0: psm[:, 0:w], h1: psm[:, 512:512 + w]}
                    else:
                        pss = {h: psS.tile([128, S], f32, tag="sc",
                                           name=f"sc{h}_{m}")[:, 0:w]
                               for h in (h0, h1)}
                    for h in (h0, h1):
                        # causal mask for the diagonal block: M^T @ I adds
                        # -60 above the diagonal, so exp() zeroes it.
                        nc.tensor.matmul(
                            pss[h][:, 0:128], lhsT=sb_mts, rhs=sb_id,
                            start=True, stop=False, skip_group_check=True)
                    for n0 in range(0, w, 512):
                        nw = min(512, w - n0)
                        for h in (h0, h1):
                            b = (h % 2) * 64
                            if n0 == 0:
                                nc.tensor.matmul(
                                    pss[h][:, 0:128],
                                    lhsT=kd[g][b:b + 64, m * 128:(m + 1) * 128],
                                    rhs=qf[hp][b:b + 64,
                                               128 * m:128 * m + 128],
                                    start=False, stop=True,
                                    skip_group_check=True)
                                if nw > 128:
                                    nc.tensor.matmul(
                                        pss[h][:, 128:nw],
                                        lhsT=kd[g][b:b + 64,
                                                   m * 128:(m + 1) * 128],
                                        rhs=qf[hp][b:b + 64,
                                                   128 * m + 128:128 * m + nw],
                                        start=True, stop=True,
                                        skip_group_check=True)
                            else:
                                nc.tensor.matmul(
                                    pss[h][:, n0:n0 + nw],
                                    lhsT=kd[g][b:b + 64, m * 128:(m + 1) * 128],
                                    rhs=qf[hp][b:b + 64,
                                               128 * m + n0:128 * m + n0 + nw],
                                    start=True, stop=True,
                                    skip_group_check=True)
                    if merged:
                        pmm = pP.tile([128, 2, 512], bf16, tag=f"P{m}",
                                      name=f"P{hp}_{m}", bufs=2)
                        nc.scalar.activation(
                            pmm[:, :, 0:w],
                            psm.rearrange("p (t x) -> p t x", x=512)[:, :, 0:w],
                            Exp, scale=sk_col)
                        Ppair[h0].append(pmm[:, 0, :][:, 0:w])
                        Ppair[h1].append(pmm[:, 1, :][:, 0:w])
                    else:
                        for h in (h0, h1):
                            # 2 allocs per pair -> ring 4 = two pairs deep
                            pm = pP.tile([128, S], bf16, tag=f"Pb{m}",
                                         name=f"P{h}_{m}", bufs=4)
                            nc.scalar.activation(pm[:, 0:w], pss[h], Exp,
                                                 scale=sk_col)
                            Ppair[h].append(pm[:, 0:w])
                return Ppair

            def attn_pv(hp, Ppair):
                h0, h1 = 2 * hp, 2 * hp + 1
                g = h0 // 4
                ytc = ytp.tile([128, S], bf16, tag="ytp", name=f"ytp{hp}")
                for h in (h0, h1):
                    P = Ppair[h]
                    yh = stg.tile([65, S], bf16, tag="yh", name=f"yh{h}")
                    for j in range(2):
                        py = psY.tile([65, 512], f32, tag="y",
                                      name=f"py{h}_{j}")
                        for m in range(4 * j + 4):
                            if m <= 4 * j:
                                o0, c0, nw = 0, 512 * j - 128 * m, 512
                            else:
                                o0 = 128 * m - 512 * j
                                c0, nw = 0, 512 - o0
                            nc.tensor.matmul(
                                py[:, o0:o0 + nw],
                                lhsT=vsb[m][:, 65 * g:65 * g + 65],
                                rhs=P[m][:, c0:c0 + nw],
                                start=(m == 0), stop=(m == 4 * j + 3),
                                skip_group_check=True)
                        nc.vector.tensor_copy(yh[:, j * 512:(j + 1) * 512], py)
                    # last pair: exp stream is over, use the fast HWDGE queue
                    deng = nc.gpsimd if hp < 7 else nc.scalar
                    deng.dma_start(out=ytc[(h % 2) * 64:(h % 2) * 64 + 64, :],
                                   in_=yh[0:64, :])
                    r0 = 64 * (h % 2) + 8 * (h // 2)
                    deng.dma_start(out=s128[r0:r0 + 8, :], in_=yh[64:65, :])
                return ytc

            def normA(hp):
                """recip of denominators + row flatten (no PE work)."""
                b32 = 32 * (hp // 4)
                o8 = 8 * (hp % 4)
                rc = rcp.tile([64, 128], f32, tag="rc", name=f"rc{hp}")
                nc.vector.tensor_copy(rc[0:32, :], s128[b32:b32 + 32, :])
                nc.vector.tensor_copy(rc[32:64, :],
                                      s128[64 + b32:64 + b32 + 32, :])
                rr = rcp.tile([64, 128], f32, tag="rr", name=f"rr{hp}")
                nc.vector.reciprocal_approx_fast(rr, rc)
                rnt = rcp.tile([2, S], f32r, tag="rnt", name=f"rnt{hp}")
                # keep these OFF the scalar queue: a data-dependent DMA there
                # stalls the Activation sequencer mid-exp-stream.
                nc.sync.dma_start(
                    out=rnt[0:1, :], in_=rr[o8:o8 + 8, :].bitcast(f32r))
                nc.gpsimd.dma_start(
                    out=rnt[1:2, :],
                    in_=rr[32 + o8:32 + o8 + 8, :].bitcast(f32r))
                return rnt

            def normB(hp, ytc, rnt):
                """broadcast, apply, hi/lo fp8 split of y^T."""
                for n in range(2):
                    pb = psY.tile([128, 512], f32, tag="y",
                                  name=f"nb{hp}_{n}")
                    nc.tensor.matmul(
                        pb, lhsT=sb_b2,
                        rhs=rnt[:, n * 512:(n + 1) * 512],
                        start=True, stop=True)
                    nc.vector.tensor_mul(ytc[:, n * 512:(n + 1) * 512],
                                         ytc[:, n * 512:(n + 1) * 512], pb)
                # hi/lo split: per dc layout [lo | hi]
                nc.vector.tensor_copy(yv8[:, hp, 1, :], ytc)
                nc.vector.tensor_sub(yv8[:, hp, 0, :], ytc, yv8[:, hp, 1, :])

            # scores run two pairs ahead of PV so the exp stream never waits
            # on PV.  normA(hp) (recip + rnt DMA launch, no PE) fires right
            # after PV(hp); normB(hp) (pb matmul + apply + split) runs one
            # iteration later when its rnt has landed — neither the PE nor
            # the DVE queue ever blocks on a norm chain in flight.
            P = {0: attn_scores(0), 1: attn_scores(1)}
            prev = None
            rnts = {}
            for hp in range(8):
                if hp + 2 < 8:
                    P[hp + 2] = attn_scores(hp + 2)
                ytc = attn_pv(hp, P.pop(hp))
                rnts[hp] = normA(hp)
                if prev is not None:
                    normB(hp - 1, prev, rnts.pop(hp - 1))
                prev = ytc
            for i in range(20):
                wt = psS.tile([128, S], f32, tag="sc", name=f"warm{i}")
                nc.tensor.matmul(wt[:, 0:512], lhsT=sb_id,
                                 rhs=sb_aux[:, 0:512], start=True, stop=True)
            normB(7, prev, rnts.pop(7))

            s2.close()

            # ---------------- stage 3: output projection (fp8) --------------
            with ExitStack() as s3:
                psO = s3.enter_context(tc.tile_pool(name="psO", bufs=4,
                                                    space="PSUM"))
                osb = s3.enter_context(tc.tile_pool(name="osb", bufs=1)).tile(
                    [128, NC * DIM], bf16, tag="osb", name="osb_all")
                wpv = sb_wp.rearrange("p (dc two d) -> p dc two d",
                                      two=2, d=DIM)
                yv = y_out.rearrange("(t p) d -> p t d", p=128)
                ov = osb.rearrange("p (t d) -> p t d", d=DIM)
                def opA(po, t, n):
                    # contributions from pairs 0..5 (normalized long ago)
                    for dc in range(0, 6, 2):
                        nc.tensor.matmul(
                            po,
                            lhsT=yv8[:, dc:dc + 2, 1, t * 128:(t + 1) * 128],
                            rhs=wpv[:, dc:dc + 2, 0, n * 512:(n + 1) * 512],
                            start=(dc == 0), stop=False, perf_mode=DR)
                    for dc in range(6):
                        nc.tensor.matmul(
                            po,
                            lhsT=yv8[:, dc, :, t * 128:(t + 1) * 128],
                            rhs=wpv[:, dc, :, n * 512:(n + 1) * 512],
                            start=False, stop=False, perf_mode=DR)

                def opB(po, t, n):
                    nc.tensor.matmul(
                        po, lhsT=yv8[:, 6:8, 1, t * 128:(t + 1) * 128],
                        rhs=wpv[:, 6:8, 0, n * 512:(n + 1) * 512],
                        start=False, stop=False, perf_mode=DR)
                    for dc in (6, 7):
                        nc.tensor.matmul(
                            po,
                            lhsT=yv8[:, dc, :, t * 128:(t + 1) * 128],
                            rhs=wpv[:, dc, :, n * 512:(n + 1) * 512],
                            start=False, stop=(dc == 7), perf_mode=DR)
                    # alternate ACT/DVE so two drains run concurrently
                    dst = osb[:, t * DIM + n * 512:t * DIM + (n + 1) * 512]
                    if (2 * t + n) % 2 == 0:
                        nc.scalar.activation(dst, po, Copy, scale=1.0 / WSC)
                    else:
                        nc.vector.tensor_scalar_mul(dst, po, 1.0 / WSC)
                    if n == 1:
                        eng = nc.sync if t % 2 == 0 else nc.scalar
                        eng.dma_start(out=yv[:, t:t + 1, :],
                                      in_=ov[:, t:t + 1, :])

                groups = [(t, n) for t in range(NC) for n in range(2)]
                live = []
                for t, n in groups:
                    po = psO.tile([128, 512], f32, tag="o",
                                  name=f"ps_o{t}_{n}")
                    opA(po, t, n)
                    live.append((po, t, n))
                    if len(live) == 4:
                        opB(*live.pop(0))
                for g in live:
                    opB(*g)
    nc.compile()
    return nc


def _split_f8(a):
    hi = a.astype(F8)
    lo = (a - hi.astype(np.float32)).astype(F8)
    return hi, lo


def _host_prep(x, Wq, Wk, Wv, Wproj, q_gain, q_scale, k_scale,
               rotary_cos, rotary_sin):
    # ---- fp8 weights: q rows then k rows, out-chunk major --------------
    wqk = np.concatenate([Wq, Wk], axis=0).astype(np.float32) * WSC
    w4 = wqk.reshape(10, 128, NC, 128)        # [oc, m, kc, p]
    hi, lo = _split_f8(w4)
    wqk_f8 = np.zeros((128, 10, NC, 2, 128), dtype=F8)
    wqk_f8[:, :, :, 0, :] = hi.transpose(3, 0, 2, 1)
    wqk_f8[:, :, :, 1, :] = lo.transpose(3, 0, 2, 1)
    wqk_f8 = np.ascontiguousarray(wqk_f8.reshape(128, 10 * NC * 256))

    wvm = (Wv.astype(np.float32) * WSC).reshape(256, NC, 128)  # [m, kc, p]
    hi, lo = _split_f8(wvm)
    wv_f8 = np.zeros((128, NC, 2, 256), dtype=F8)
    wv_f8[:, :, 0, :] = hi.transpose(2, 1, 0)
    wv_f8[:, :, 1, :] = lo.transpose(2, 1, 0)
    wv_f8 = np.ascontiguousarray(wv_f8.reshape(128, NC * 512))

    # wp lhs rows are y dims (dc chunks), rhs cols are output dims
    wpm = (Wproj.T.astype(np.float32) * WSC).reshape(NC, 128, DIM)  # [dc,p,d]
    hi, lo = _split_f8(wpm)
    wp_f8 = np.zeros((128, NC, 2, DIM), dtype=F8)
    wp_f8[:, :, 0, :] = hi.transpose(1, 0, 2)
    wp_f8[:, :, 1, :] = lo.transpose(1, 0, 2)
    wp_f8 = np.ascontiguousarray(wp_f8.reshape(128, NC * 2 * DIM))

    shared = {"wqk": wqk_f8, "wv": wv_f8, "wp": wp_f8}

    aux = np.zeros((128, A_N), dtype=np.float32)
    cos = np.asarray(rotary_cos, np.float32).reshape(B * S, HD // 2)[:S].T
    sin = np.asarray(rotary_sin, np.float32).reshape(B * S, HD // 2)[:S].T
    aux[:, A_COS:A_COS + S] = np.tile(cos, (4, 1))
    aux[:, A_SIN:A_SIN + S] = np.tile(sin, (4, 1))
    # rotate-half permutation (lhsT = R.T), exact in bf16
    R = np.zeros((128, 128), dtype=np.float32)
    for i in range(128):
        if i % 64 < 32:
            R[i, i + 32] = 1.0
        else:
            R[i, i - 32] = -1.0
    aux[:, A_RT:A_RT + 128] = R.T
    # q sum-sq indicators: head h -> row h (plain)
    for c in range(8):
        for r in range(128):
            h = 2 * c + r // 64
            aux[r, A_BSQ + 16 * c + h] = 1.0
    # identity + causal -60 seed for diagonal blocks of scores^T
    ar = np.arange(128)
    aux[:, A_ID:A_ID + 128] = np.eye(128, dtype=np.float32)
    aux[:, A_MTS:A_MTS + 128] = -60.0 * (ar[None, :] < ar[:, None]).T
    aux[:, A_ONE:A_ONE + 1] = 1.0
    shared["aux"] = aux.astype(BF)

    b2 = np.zeros((2, 128), dtype=np.float32)
    b2[0, 0:64] = 1.0
    b2[1, 64:128] = 1.0
    shared["b2"] = b2

    # q gains: row h (plain); folds q_scale and HD^-0.5
    gg = np.zeros((16, 1), dtype=np.float32)
    qg = np.asarray(q_gain, np.float32) * float(q_scale) * (HD ** -0.5)
    for h in range(H):
        gg[h, 0] = qg[h]
    shared["gains"] = gg
    assert abs(float(k_scale) - 1.0) < 1e-6, "k_scale fold not implemented"

    per_core = []
    x = np.asarray(x, np.float32)
    for c in range(B):
        xd = x[c * S:(c + 1) * S]                     # [1024 t, 1024 d]
        xT = xd.T.reshape(NC, 128, S)                 # [kc, p, t]
        hi, lo = _split_f8(xT)
        xf = np.zeros((128, NC, 2, S), dtype=F8)
        xf[:, :, 0, :] = lo.transpose(1, 0, 2)
        xf[:, :, 1, :] = hi.transpose(1, 0, 2)
        per_core.append({"xf": np.ascontiguousarray(
            xf.reshape(128, NC * 2 * S))})
    return shared, per_core


def kernel(x, Wq, Wk, Wv, Wproj, q_gain, q_scale, k_scale,
           rotary_cos, rotary_sin, cu_seqlens=None, max_doc_len=None,
           **_ignored):
    x = np.asarray(x, np.float32)
    assert x.shape == (B * S, DIM), x.shape
    if "nc" not in _CACHE:
        _CACHE["nc"] = _build()
    nc = _CACHE["nc"]
    shared, per_core = _host_prep(
        np.asarray(x, np.float32), np.asarray(Wq, np.float32),
        np.asarray(Wk, np.float32), np.asarray(Wv, np.float32),
        np.asarray(Wproj, np.float32), np.asarray(q_gain, np.float32),
        np.asarray(q_scale, np.float32), np.asarray(k_scale, np.float32),
        np.asarray(rotary_cos, np.float32), np.asarray(rotary_sin, np.float32))
    in_maps = [{**shared, **pc} for pc in per_core]
    res = bass_utils.run_bass_kernel_spmd(
        nc, in_maps, core_ids=list(range(B)),
        trace=bool(int(os.environ.get("KERNEL_TRACE", "0"))))
    _CACHE["last_results"] = res
    out = np.concatenate(
        [np.asarray(res.results[c]["y"]).astype(np.float32) for c in range(B)],
        axis=0)
    return out
